# revision 10
# baseline (speedup 1.0000x reference)
"""Bass/Trainium2 kernel for nn_DFTLayer: out[b,f,k] = DFT_1024(x[b,f,:]).

reference: real = einsum('bfs,ks->bfk', x, wcos); imag = ... wsin
           out  = complex(real, -imag),  x: [16, 1024, 1024] f32.

Strategy (8 NeuronCores, data-parallel over batch, 2 batches/core):
  - wcos/wsin are symmetric (w[k,s] == w[s,k]), so x @ w.T == x @ w.
  - Hermitian symmetry (x real): out[k] = conj(out[N-k]). The device only
    computes freq cols k = 1..512; col 0 is a host row-sum, cols 513..1023
    are a host conjugate mirror.
  - Cosine/sine parity over s (DCT/DST fold): with u[s] = x[s] + x[N-s],
    v[s] = x[s] - x[N-s] (s = 1..511), u[0] = v[0] = x[0]:
        real[k] = (U @ wcos[0:512, k]) + (-1)^k x[512]   (x[512] term on host)
        imag[k] =  V @ wsin[0:512, k]
    This halves both the matmul work and the DFT-kernel DMA.
  - U/V are built on the DVE (negative-stride reversed operand), transposed
    on the PE (128x128 blocks, 4 per PSUM bank), copied to SBUF as
    float32r, then contracted in 4 chunk-matmuls per output at N=512.
  - float32r (FP22 multiply, FP32 accumulate) runs at 1 PE cycle/row:
    4x faster than true fp32, rel err ~1.3e-4.
"""

import sys

for _p in ("/opt/trn_rl_repo", "/root/.axon_site/_ro/trn_rl_repo"):
    if _p not in sys.path:
        sys.path.append(_p)

import numpy as np
from contextlib import ExitStack

N_CORES = 8
B, F_FULL, S = 16, 1024, 1024          # x: [B, F_FULL, S]
F = (B // N_CORES) * F_FULL            # 2048 rows per core
KD = 512                               # device computes freq cols 1..512
SH = 512                               # folded contraction length (s = 0..511)
N_FT = F // 128                        # 16 row tiles per core
N_SC = SH // 128                       # 4 contraction chunks after the fold

_CACHE = {}


def _build():
    """Build + compile the per-core Bass program (cached)."""
    if "nc" in _CACHE:
        return _CACHE["nc"]

    from concourse import bacc, tile, mybir

    f32 = mybir.dt.float32
    f32r = mybir.dt.float32r

    nc = bacc.Bacc("TRN2", target_bir_lowering=False, debug=False)

    x_d = nc.dram_tensor("x", [F, S], f32, kind="ExternalInput")
    wc_d = nc.dram_tensor("wc", [SH, KD], f32, kind="ExternalInput")
    ws_d = nc.dram_tensor("ws", [SH, KD], f32, kind="ExternalInput")
    re_d = nc.dram_tensor("re", [F, KD], f32, kind="ExternalOutput")
    im_d = nc.dram_tensor("im", [F, KD], f32, kind="ExternalOutput")
    # freq col 0 (real part = full row-sum), packed [partition, f_tile]
    c0_d = nc.dram_tensor("c0", [128, N_FT], f32, kind="ExternalOutput")

    ident_d = nc.inline_tensor(np.eye(128, dtype=np.float32), name="ident")
    # alt[j] = (-1)^(j+1) for device col j <-> freq k = j+1 (x[512] term)
    alt_np = np.tile(np.where(np.arange(1, KD + 1) % 2 == 0, 1.0, -1.0)
                     .astype(np.float32), (128, 1))
    alt_d = nc.inline_tensor(alt_np, name="alt")

    with tile.TileContext(nc) as tc, ExitStack() as ctx:
        wpool = ctx.enter_context(tc.tile_pool(name="w", bufs=1))
        xpool = ctx.enter_context(tc.tile_pool(name="x", bufs=3))
        uvpool = ctx.enter_context(tc.tile_pool(name="uv", bufs=2))
        xtpool = ctx.enter_context(tc.tile_pool(name="xt", bufs=2))
        opool = ctx.enter_context(tc.tile_pool(name="o", bufs=3))
        ptpool = ctx.enter_context(tc.tile_pool(name="pt", bufs=3, space="PSUM"))
        prpool = ctx.enter_context(tc.tile_pool(name="pr", bufs=2, space="PSUM"))
        pipool = ctx.enter_context(tc.tile_pool(name="pi", bufs=2, space="PSUM"))

        # x row-tile loads; first two issued before anything else so the
        # fold/transpose pipeline starts while the DFT kernels stream in.
        x_ts = [None] * N_FT

        def load_x(ft):
            x_t = xpool.tile([128, S], f32, tag="x_t")
            nc.sync.dma_start(x_t[:], x_d[ft * 128:(ft + 1) * 128, :])
            x_ts[ft] = x_t

        load_x(0)
        load_x(1)

        ident = wpool.tile([128, 128], f32r)
        nc.sync.dma_start(ident[:], ident_d[:].bitcast(f32r))
        alt_t = wpool.tile([128, KD], f32)
        nc.sync.dma_start(alt_t[:], alt_d[:])
        c0_acc = wpool.tile([128, N_FT], f32)   # col-0 row-sums, one col/f_tile
        x5_acc = wpool.tile([128, N_FT], f32)   # x[:, 512] stash, one col/f_tile

        # Folded DFT kernels (rows s = 0..511), resident for the whole
        # run; one tile + DMA per 128-row chunk, in consumption order.
        wc_r = wc_d[:].rearrange("(c p) j -> p c j", p=128).bitcast(f32r)
        ws_r = ws_d[:].rearrange("(c p) j -> p c j", p=128).bitcast(f32r)
        wc_ts, ws_ts = [], []
        for c in range(N_SC):
            wc_t = wpool.tile([128, KD], f32r, tag=f"wc{c}")
            nc.sync.dma_start(wc_t[:], wc_r[:, c, :])
            wc_ts.append(wc_t)
            ws_t = wpool.tile([128, KD], f32r, tag=f"ws{c}")
            nc.sync.dma_start(ws_t[:], ws_r[:, c, :])
            ws_ts.append(ws_t)

        uvts = [None] * N_FT

        def fold_and_transpose(ft):
            x_t = x_ts[ft]
            # u = x[s] + x[1024-s], v = x[s] - x[1024-s]  (s = 1..511);
            # col 0 carries x[0] (cos row 0 == 1, sin row 0 == 0).
            # The U add also accumulates sum_{s=1..511} u[s] (accum_out),
            # from which freq col 0 = accum + x[0] + x[512].
            u_t = uvpool.tile([128, SH], f32r, tag="u")
            c0p = uvpool.tile([128, 1], f32, tag="c0p")
            nc.vector.tensor_copy(u_t[:, 0:1], x_t[:, 0:1])
            nc.vector.scalar_tensor_tensor(
                u_t[:, 1:SH], x_t[:, 1:SH], 1.0, x_t[:, S - 1:SH:-1],
                op0=mybir.AluOpType.mult, op1=mybir.AluOpType.add,
                accum_out=c0p[:],
            )
            t0 = uvpool.tile([128, 1], f32, tag="t0")
            nc.vector.tensor_add(t0[:], x_t[:, 0:1], x_t[:, 512:513])
            nc.vector.tensor_add(c0_acc[:, ft:ft + 1], t0[:], c0p[:])
            nc.vector.tensor_copy(x5_acc[:, ft:ft + 1], x_t[:, 512:513])
            v_t = uvpool.tile([128, SH], f32r, tag="v")
            nc.vector.tensor_copy(v_t[:, 0:1], x_t[:, 0:1])
            nc.vector.tensor_sub(v_t[:, 1:SH], x_t[:, 1:SH], x_t[:, S - 1:SH:-1])
            # transpose U and V 128 cols at a time: uvt[:, c, :] holds
            # U chunks (c = 0..3) then V chunks (c = 4..7)
            uvt = xtpool.tile([128, 2 * N_SC, 128], f32r)
            for g, src in ((0, u_t), (1, v_t)):
                pt = ptpool.tile([128, N_SC, 128], f32r)
                for c in range(N_SC):
                    nc.tensor.matmul(
                        pt[:, c, :],
                        src[:, c * 128:(c + 1) * 128],
                        ident[:],
                        is_transpose=True,
                        start=(c == 0),
                        stop=(c == N_SC - 1),
                    )
                if g == 0:
                    nc.scalar.copy(uvt[:, 0:N_SC, :], pt[:])
                else:
                    nc.scalar.copy(uvt[:, N_SC:2 * N_SC, :], pt[:])
            uvts[ft] = uvt

        def matmul_and_store(ft):
            uvt = uvts[ft]
            ps_re = prpool.tile([128, KD], f32)
            for c in range(N_SC):
                nc.tensor.matmul(ps_re[:], uvt[:, c, :], wc_ts[c][:],
                                 start=(c == 0), stop=(c == N_SC - 1))
            ps_im = pipool.tile([128, KD], f32)
            for c in range(N_SC):
                nc.tensor.matmul(ps_im[:], uvt[:, N_SC + c, :], ws_ts[c][:],
                                 start=(c == 0), stop=(c == N_SC - 1))
            # real with the fold edge term: re = ps_re + alt * x[:, 512]
            nsplit = 2 if ft == N_FT - 1 else 1
            w = KD // nsplit
            re_sb = opool.tile([128, KD], f32)
            im_sb = opool.tile([128, KD], f32)
            for h in range(nsplit):
                sl = slice(h * w, (h + 1) * w)
                nc.vector.scalar_tensor_tensor(
                    re_sb[:, sl], alt_t[:, sl], x5_acc[:, ft:ft + 1], ps_re[:, sl],
                    op0=mybir.AluOpType.mult, op1=mybir.AluOpType.add,
                )
                nc.sync.dma_start(re_d[ft * 128:(ft + 1) * 128, sl], re_sb[:, sl])
                # negate imag on the way out: out.imag = -(v @ wsin)
                nc.scalar.mul(im_sb[:, sl], ps_im[:, sl], -1.0)
                nc.sync.dma_start(im_d[ft * 128:(ft + 1) * 128, sl], im_sb[:, sl])

        # Software pipeline: fold+transposes of ft+1 hit the PE queue
        # before the matmuls of ft, so the PE never waits on the
        # DVE/ACT fold+copy chain.
        fold_and_transpose(0)
        for ft in range(1, N_FT):
            if ft + 1 < N_FT:
                load_x(ft + 1)
            fold_and_transpose(ft)
            matmul_and_store(ft - 1)
        matmul_and_store(N_FT - 1)
        nc.sync.dma_start(c0_d[:], c0_acc[:])

    nc.compile()
    _CACHE["nc"] = nc
    return nc


def kernel(x, wsin, wcos):
    from concourse.bass_utils import run_bass_kernel_spmd

    x = np.asarray(x, dtype=np.float32)
    wsin = np.asarray(wsin, dtype=np.float32)
    wcos = np.asarray(wcos, dtype=np.float32)

    nc = _build()

    # By symmetry w[k, s] == w[s, k]: rows 0..511, freq cols 1..512.
    wc = np.ascontiguousarray(wcos[0:SH, 1:KD + 1])
    ws = np.ascontiguousarray(wsin[0:SH, 1:KD + 1])

    bpc = B // N_CORES
    in_maps = [
        {"x": np.ascontiguousarray(x[c * bpc:(c + 1) * bpc].reshape(F, S)),
         "wc": wc, "ws": ws}
        for c in range(N_CORES)
    ]

    res = run_bass_kernel_spmd(
        nc, in_maps, core_ids=list(range(N_CORES)), **_CACHE.get("run_kwargs", {})
    )
    kernel.last_results = res

    out = np.empty((B, F_FULL, S), dtype=np.complex64)
    fv = out.view(np.float32).reshape(B, F_FULL, 2 * S)
    for c in range(N_CORES):
        b0 = c * bpc
        re = res.results[c]["re"].reshape(bpc, F_FULL, KD)
        im = res.results[c]["im"].reshape(bpc, F_FULL, KD)  # already -imag
        blk = fv[b0:b0 + bpc]
        # col 0: real = row-sum of x (cos(0)=1), imag = 0 (sin(0)=0);
        # c0 is packed [partition, f_tile] -> row 128*ft + p
        blk[:, :, 0] = res.results[c]["c0"].T.reshape(bpc, F_FULL)
        blk[:, :, 1] = 0.0
        blk[:, :, 2:2 * KD + 2:2] = re          # real, k = 1..512
        blk[:, :, 3:2 * KD + 3:2] = im          # imag, k = 1..512
        # Hermitian mirror: out[k] = conj(out[1024-k]) for k = 513..1023
        blk[:, :, 2 * KD + 2::2] = re[:, :, KD - 2::-1]
        blk[:, :, 2 * KD + 3::2] = -im[:, :, KD - 2::-1]
    return out
